# revision 4
# baseline (speedup 1.0000x reference)
"""NeighborSample Trainium2 kernel — halo-replicated layout (v3).

Input  x:   (8, 64, 64, 192) f32
Output:     (8*64*64, 5, 5, 192) f32 — out[b*4096 + h*64 + w, i, j, c] =
            x[b, h+i-2, w+j-2, c] (zero-padded).

Engine SBUF access patterns must start at partition 0/32/64/96 (BIR verifier
rule), so partition-shifted engine copies are illegal. Instead each SBUF
partition holds ALL the input data needed for its own pixels (halo
replication, 3x read amplification), so patch materialization is
same-partition copies only:

  - Host pads x to xp [68, 68, 192] (zero halo) — no on-chip memsets needed.
  - buf7 [128, 18432]: partition p = g*8 + q holds padded rows [4g, 4g+8) x
    padded cols [8q, 8q+12) (layout [rr, cc, c]). Loaded by 8 DMAs (one per
    rr) with 9216 B descriptors; 9.4 MB total reads.
  - Pixel (h, w) = (4g + hl, 8q + wl) lives on partition p. 16 rounds, one
    per (hl, wlp): DVE (i=0,1), ACT (i=2,3), Pool (i=4) copy
      pb[p, qq*4800 + i*960 + :960] = buf7[p, (hl+i)*2304 + (2*wlp+qq)*192 + :960]
    building two adjacent pixels' contiguous patches (9600 f32) per partition.
  - One store DMA per round: 128 descriptors x 38400 B (vs 3840 B in v1).

Synchronization (all waits must tolerate out-of-order DMA completion, so no
cumulative waits over >1 outstanding DMA per semaphore):
  - one semaphore per load DMA; copies for round (hl, *) wait loads rr<=hl+4
    (incrementally, engine program order covers earlier rr).
  - per-engine copy semaphores; store t waits cs_dve>=2(t+1), cs_act>=2(t+1),
    cs_pool>=t+1.
  - two store semaphores by round parity (<=1 outstanding each); copies of
    round t>=2 wait store_sems[t%2] >= 16*(t//2) (store t-2 done, pb reuse).
"""

import sys
from contextlib import ExitStack

for _p in ("/opt/trn_rl_repo",):
    if _p not in sys.path:
        sys.path.insert(0, _p)

import numpy as np

import concourse.bass as bass
import concourse.mybir as mybir
from concourse.bass_utils import run_bass_kernel_spmd

B = 8
H = W = 64
C = 192
K = 5
PAD = 2
G = 4                    # pixel rows per partition group
WC = 8                   # pixel cols per partition chunk
NG = H // G              # 16 groups
NQ = W // WC             # 8 chunks -> 128 partitions, p = g*8 + q
RR = G + 2 * PAD         # 8 buf rows per partition
CC = WC + 2 * PAD        # 12 buf cols per partition
BCOL = CC * C            # 2304 elems per buf row
BROW = RR * BCOL         # 18432 elems per buf partition
PXROW = (W + 2 * PAD) * C  # 13056 elems per padded-input row
WIN = K * C              # 960
PATCH = K * K * C        # 4800
OUT_H = W * PATCH        # 307200
WBL = 2                  # pixels (adjacent w) per partition per round
PBROW = WBL * PATCH      # 9600
NWLP = WC // WBL         # 4 wl-pairs
NROUND = G * NWLP        # 16 rounds


def build_nc() -> bass.Bass:
    nc = bass.Bass()
    xp = nc.declare_dram_parameter(
        "xp", [H + 2 * PAD, W + 2 * PAD, C], mybir.dt.float32, isOutput=False
    )
    out = nc.declare_dram_parameter(
        "out", [H, W, K, K, C], mybir.dt.float32, isOutput=True
    )

    with ExitStack() as ctx:
        block = ctx.enter_context(nc.Block())
        load_sems = [
            ctx.enter_context(nc.semaphore(f"load{rr}")) for rr in range(RR)
        ]
        cs = {
            e: ctx.enter_context(nc.semaphore(f"cs_{e}"))
            for e in ("v", "a", "p")
        }
        store_sems = [
            ctx.enter_context(nc.semaphore(f"ss{par}")) for par in range(2)
        ]
        buf7 = ctx.enter_context(
            nc.sbuf_tensor("buf7", [128, BROW], mybir.dt.float32)
        )
        pbs = [
            ctx.enter_context(
                nc.sbuf_tensor(f"pb{par}", [128, PBROW], mybir.dt.float32)
            )
            for par in range(2)
        ]

        rounds = [(hl, wlp) for hl in range(G) for wlp in range(NWLP)]

        def emit_copies(eng, my_sem, idxs, do_copy):
            waited_rr = -1
            for t, (hl, wlp) in enumerate(rounds):
                while waited_rr < hl + K - 1:
                    waited_rr += 1
                    eng.wait_ge(load_sems[waited_rr], 16)
                if t >= 2:
                    eng.wait_ge(store_sems[t % 2], 16 * (t // 2))
                pb = pbs[t % 2]
                for i in idxs:
                    do_copy(
                        bass.AP(pb, i * WIN, [[PBROW, 128], [PATCH, WBL], [1, WIN]]),
                        bass.AP(
                            buf7,
                            (hl + i) * BCOL + wlp * WBL * C,
                            [[BROW, 128], [C, WBL], [1, WIN]],
                        ),
                    ).then_inc(my_sem, 1)

        @block.vector
        def _(vector):
            emit_copies(
                vector, cs["v"], (0, 1), lambda o, i: vector.tensor_copy(out=o, in_=i)
            )

        @block.scalar
        def _(scalar):
            # loads on the ACT ring: one DMA per rr slice of all partitions
            for rr in range(RR):
                scalar.dma_start(
                    out=bass.AP(buf7, rr * BCOL, [[BROW, 128], [1, BCOL]]),
                    in_=bass.AP(
                        xp,
                        rr * PXROW,
                        [[G * PXROW, NG], [WC * C, NQ], [1, BCOL]],
                    ),
                ).then_inc(load_sems[rr], 16)
            emit_copies(
                scalar, cs["a"], (2, 3), lambda o, i: scalar.copy(out=o, in_=i)
            )

        @block.gpsimd
        def _(gpsimd):
            emit_copies(
                gpsimd, cs["p"], (4,), lambda o, i: gpsimd.tensor_copy(out=o, in_=i)
            )

        @block.sync
        def _(sync):
            for t, (hl, wlp) in enumerate(rounds):
                sync.wait_ge(cs["v"], 2 * (t + 1))
                sync.wait_ge(cs["a"], 2 * (t + 1))
                sync.wait_ge(cs["p"], t + 1)
                sync.dma_start(
                    out=bass.AP(
                        out,
                        hl * OUT_H + wlp * WBL * PATCH,
                        [[G * OUT_H, NG], [WC * PATCH, NQ], [1, PBROW]],
                    ),
                    in_=bass.AP(pbs[t % 2], 0, [[PBROW, 128], [1, PBROW]]),
                ).then_inc(store_sems[t % 2], 16)
            sync.wait_ge(store_sems[0], 16 * (NROUND // 2))
            sync.wait_ge(store_sems[1], 16 * (NROUND // 2))

    return nc


_NC_CACHE = None


def make_in_maps(x):
    return [
        {"xp": np.pad(x[i], ((PAD, PAD), (PAD, PAD), (0, 0)))} for i in range(B)
    ]


def kernel(x) -> np.ndarray:
    global _NC_CACHE
    x = np.asarray(x, dtype=np.float32)
    assert x.shape == (B, H, W, C), x.shape
    if _NC_CACHE is None:
        _NC_CACHE = build_nc()
    in_maps = make_in_maps(x)
    res = run_bass_kernel_spmd(_NC_CACHE, in_maps, list(range(B)))
    outs = [res.results[i]["out"].reshape(H * W, K, K, C) for i in range(B)]
    return np.concatenate(outs, axis=0)


# revision 5
# speedup vs baseline: 1.0205x; 1.0205x over previous
"""NeighborSample Trainium2 kernel — pure-DMA, 3-ring streaming (v5).

Input  x:   (8, 64, 64, 192) f32
Output:     (8*64*64, 5, 5, 192) f32 — out[b*4096 + h*64 + w, i, j, c] =
            x[b, h+i-2, w+j-2, c] (zero-padded).

Trace analysis of v1 (two rings, 3840 B descriptors) and v3 (one ring,
38400 B descriptors from materialized patches) showed:
  - a single SDMA engine moves a 3840 B descriptor in ~149 ns (25.8 GB/s)
    when its descriptor feed is deep, because the engine pipelines the SBUF
    read of descriptor n+1 with the HBM write of descriptor n;
  - one large descriptor serializes its own read/write -> only ~14.5 GB/s;
  - v1 averaged 17.5 GB/s/engine because the feed had gaps (2 rings, uneven
    14/15-row segments with <16-engine fan-out, serial zero-row stores).

v5 therefore keeps the v1 dataflow (sliding-window source APs, 3840 B
descriptors, one descriptor per (h, w, i)) and fixes the feed:
  - host-pads x to xp [68, 68, 192]: buf partitions hold padded rows, so
    every store DMA covers a full uniform [32 h x 32 w x 960] block with
    16-engine fan-out; no memsets, no zero-row stores, no segment tails.
  - bufL/bufR [68 partitions x 6912]: padded rows, left half cols [0, 36),
    right half cols [32, 68). Loaded by 4 DMAs (27648 B descriptors).
  - 20 store DMAs (2 halves x 5 i x 2 h-segments), round-robined over THREE
    descriptor-generation rings: SP HWDGE, ACT HWDGE, Pool SWDGE. Three
    independent streams keep all 16 engines fed continuously.
  - no synchronization at all between stores; each ring waits once for the
    4 loads, then streams.
"""

import sys

for _p in ("/opt/trn_rl_repo",):
    if _p not in sys.path:
        sys.path.insert(0, _p)

import numpy as np

import concourse.bass as bass
import concourse.mybir as mybir
from concourse.bass_utils import run_bass_kernel_spmd

B = 8
H = W = 64
C = 192
K = 5
PAD = 2
HALF = 32                # w positions per half
COLS = 36                # cols per half buffer
ROW = COLS * C           # 6912 elems per buf partition
PROWS = H + 2 * PAD      # 68 padded rows
PXROW = PROWS * C        # 13056 elems per padded-input row
WIN = K * C              # 960 (3840 B descriptor)
OUT_W = K * K * C        # 4800
OUT_H = W * OUT_W        # 307200
HSEG = 32                # h rows per store DMA


def build_nc() -> bass.Bass:
    nc = bass.Bass()
    xp = nc.declare_dram_parameter(
        "xp", [PROWS, PROWS, C], mybir.dt.float32, isOutput=False
    )
    out = nc.declare_dram_parameter(
        "out", [H, W, K, K, C], mybir.dt.float32, isOutput=True
    )

    with (
        nc.Block() as block,
        nc.semaphore("lm0") as lm0,
        nc.semaphore("lm1") as lm1,
        nc.semaphore("lt0") as lt0,
        nc.semaphore("lt1") as lt1,
        nc.semaphore("sS") as sS,
        nc.semaphore("sA") as sA,
        nc.semaphore("sP") as sP,
        nc.sbuf_tensor("bufL", [128, ROW], mybir.dt.float32) as bufL,
        nc.sbuf_tensor("bufR", [128, ROW], mybir.dt.float32) as bufR,
    ):
        bufs = [bufL, bufR]
        load_sems = [lm0, lm1, lt0, lt1]

        # store work list: (half, i, hseg) round-robined over 3 rings
        jobs = [
            (s, i, g) for i in range(K) for g in range(2) for s in range(2)
        ]
        rings = {0: [], 1: [], 2: []}
        for k, job in enumerate(jobs):
            rings[k % 3].append(job)

        def emit_loads(eng, which):
            # which: 0 -> left main+tail, 1 -> right main+tail
            s = which
            col0 = s * HALF * C
            eng.dma_start(
                out=bass.AP(bufs[s], 0, [[ROW, 64], [1, ROW]]),
                in_=bass.AP(xp, col0, [[PXROW, 64], [1, ROW]]),
            ).then_inc(load_sems[s], 16)
            eng.dma_start(
                out=bass.AP(bufs[s], 64 * ROW, [[ROW, 4], [1, ROW]]),
                in_=bass.AP(xp, 64 * PXROW + col0, [[PXROW, 4], [1, ROW]]),
            ).then_inc(load_sems[2 + s], 16)

        def emit_stores(eng, my_sem, my_jobs):
            for sem in load_sems:
                eng.wait_ge(sem, 16)
            for s, i, g in my_jobs:
                eng.dma_start(
                    out=bass.AP(
                        out,
                        g * HSEG * OUT_H + s * HALF * OUT_W + i * WIN,
                        [[OUT_H, HSEG], [OUT_W, HALF], [1, WIN]],
                    ),
                    in_=bass.AP(
                        bufs[s],
                        (i + g * HSEG) * ROW,
                        [[ROW, HSEG], [C, HALF], [1, WIN]],
                    ),
                ).then_inc(my_sem, 16)
            eng.wait_ge(my_sem, 16 * len(my_jobs))

        @block.sync
        def _(sync):
            emit_loads(sync, 0)
            emit_stores(sync, sS, rings[0])

        @block.scalar
        def _(scalar):
            emit_loads(scalar, 1)
            emit_stores(scalar, sA, rings[1])

        @block.gpsimd
        def _(gpsimd):
            emit_stores(gpsimd, sP, rings[2])

    return nc


_NC_CACHE = None


def make_in_maps(x):
    return [
        {"xp": np.pad(x[i], ((PAD, PAD), (PAD, PAD), (0, 0)))} for i in range(B)
    ]


def kernel(x) -> np.ndarray:
    global _NC_CACHE
    x = np.asarray(x, dtype=np.float32)
    assert x.shape == (B, H, W, C), x.shape
    if _NC_CACHE is None:
        _NC_CACHE = build_nc()
    in_maps = make_in_maps(x)
    res = run_bass_kernel_spmd(_NC_CACHE, in_maps, list(range(B)))
    outs = [res.results[i]["out"].reshape(H * W, K, K, C) for i in range(B)]
    return np.concatenate(outs, axis=0)
